# revision 10
# baseline (speedup 1.0000x reference)
"""Trainium2 Bass kernel for nn_Analytic_net (gnn_message_passing).

Computes: A = constant_part + einsum('eij,e->ij', M, r);
          out = solve(A, [zeros(500, P); x.reshape(12, P)])   # (512, 4096)

Distribution (8 NeuronCores): M and r sharded along E (128 edges/core);
each core computes a partial einsum via column-tiled M=32 f32 matmuls,
AllReduces the (512,512) partial, then runs a replicated dense solve:

  shift-regularized scaled Newton-Schulz inverse iteration
    (U = c2*A@A^T + eta*I tracked with Xt = X^T; all lhsT operands are
     symmetric so no per-iteration transposes), followed by
  CG on V = A@X (SPD, clustered spectrum) for the 12 relevant columns
    (rhs = I[:, 500:512]), Y = X@z, and out_shard = Y @ e_shard.

Self-contained: hardcodes all shapes; builds host-side constants inline.
"""
import numpy as np

NCORES = 8
E, N, MD, NPTS = 1024, 512, 12, 4096
ESH = E // NCORES            # 128 edges per core
JSH = NPTS // NCORES         # 512 points per core
NB = N // 128                # 4 blocks of 128
NGRP = N // 4                # einsum groups of 4 rows

ETA = 1e-5                   # NS shift regularization
MARGIN2 = 1.5625             # (1.25 margin)^2 on sigma_max estimate
CG_ITERS = 18
NS_POLISH = 2

_F = np.float32


def _schedule():
    l = ETA * 0.5
    gs = []
    while l < 0.7:
        gs.append(2.0 / (1.0 + l))
        l = 4.0 * l / (1.0 + l) ** 2
    return gs + [1.0] * NS_POLISH


def _block_layout(Z):
    """(512,512) -> (128, 16*128); block (bi,bj) at cols (bi*4+bj)*128."""
    return np.ascontiguousarray(
        Z.reshape(4, 128, 4, 128).transpose(1, 0, 2, 3).reshape(128, 2048)
    ).astype(_F)


def _vec_layout(V):
    """(512,w) -> (128, 4*w); block bi at cols bi*w."""
    w = V.shape[1]
    return np.ascontiguousarray(
        V.reshape(4, 128, w).transpose(1, 0, 2).reshape(128, 4 * w)
    ).astype(_F)


def _build(solve_only=False, dbg=False):
    import concourse.bacc as bacc
    import concourse.tile as tile
    import concourse.mybir as mybir

    dt = mybir.dt.float32
    ncores = 1 if solve_only else NCORES
    nc = bacc.Bacc("TRN2", target_bir_lowering=False, debug=False,
                   num_devices=ncores)
    dbg_d = {}
    if dbg:
        for nm, shp in [("dbg_c2", [128, 1]), ("dbg_U", [128, 2048]),
                        ("dbg_Xt", [128, 2048]), ("dbg_V", [128, 2048]),
                        ("dbg_Z", [128, 48]), ("dbg_At", [128, 2048]),
                        ("dbg_R", [128, 48]), ("dbg_Pc", [128, 48])]:
            dbg_d[nm] = nc.dram_tensor(nm, shp, dt, kind="ExternalOutput")
    if solve_only:
        Afull_d = nc.dram_tensor("Afull", [N, N], dt, kind="ExternalInput")
    else:
        M_d = nc.dram_tensor("M", [ESH, N, N], dt, kind="ExternalInput")
        rrep_d = nc.dram_tensor("r_rep", [ESH, 32], dt, kind="ExternalInput")
        C_d = nc.dram_tensor("C", [N, N], dt, kind="ExternalInput")
    e_d = nc.dram_tensor("e", [MD, JSH], dt, kind="ExternalInput")
    I2_d = nc.dram_tensor("I2", [128, 2048], dt, kind="ExternalInput")
    EI_d = nc.dram_tensor("EI", [128, 2048], dt, kind="ExternalInput")
    ID_d = nc.dram_tensor("ID", [128, 128], dt, kind="ExternalInput")
    B_d = nc.dram_tensor("B", [128, 4 * MD], dt, kind="ExternalInput")
    out_d = nc.dram_tensor("out", [N, JSH], dt, kind="ExternalOutput")

    add = mybir.AluOpType.add
    sub = mybir.AluOpType.subtract
    mult = mybir.AluOpType.mult

    gammas = _schedule()

    with tile.TileContext(nc) as tc:
        with tc.tile_pool(name="dram", bufs=1, space="DRAM") as dram, \
             tc.tile_pool(name="consts", bufs=1) as cp, \
             tc.tile_pool(name="mats", bufs=1) as mats, \
             tc.tile_pool(name="work", bufs=2) as work, \
             tc.tile_pool(name="small", bufs=2) as sp:

            A_sb = mats.tile([128, 2048], dt, tag="A")
            if solve_only:
                for bi in range(NB):
                    nc.sync.dma_start(A_sb[:, bi * N:(bi + 1) * N],
                                      Afull_d[128 * bi:128 * (bi + 1), :])
            else:
                # ---------------- Phase E: einsum partial ----------------
                partial_b = dram.tile([N, N], dt)
                ar_b = dram.tile([N, N], dt)
                rt = cp.tile([ESH, 32], dt)
                nc.sync.dma_start(rt[:], rrep_d[:])
                with tc.tile_pool(name="einsum", bufs=3) as ep, \
                     tc.tile_pool(name="edrain", bufs=4) as edp, \
                     tc.tile_pool(name="psumE", bufs=8, space="PSUM") as ppe:
                    for g in range(NGRP):
                        mt = ep.tile([ESH, 4 * N], dt, tag="mt")
                        nc.sync.dma_start(mt[:], M_d[:, 4 * g:4 * g + 4, :])
                        pt = ppe.tile([128, N], dt, tag="pt")
                        for c in range(4):
                            nc.tensor.matmul(
                                pt[32 * c:32 * c + 32, :], lhsT=rt[:],
                                rhs=mt[:, c * N:(c + 1) * N],
                                start=True, stop=True, tile_position=(0, 32 * c))
                        sc = edp.tile([128, N], dt, tag="sc")
                        nc.scalar.copy(sc[:], pt[:])
                        nc.sync.dma_start(partial_b[4 * g:4 * g + 4, :],
                                          sc[0:128:32, :])

                # ------------- Phase R: AllReduce + A assembly -----------
                nc.gpsimd.collective_compute(
                    "AllReduce", add, replica_groups=[list(range(NCORES))],
                    ins=[partial_b.opt()], outs=[ar_b.opt()])

                for bi in range(NB):
                    art = work.tile([128, N], dt, tag="art")
                    ct = work.tile([128, N], dt, tag="ct")
                    nc.sync.dma_start(art[:], ar_b[128 * bi:128 * (bi + 1), :])
                    nc.sync.dma_start(ct[:], C_d[128 * bi:128 * (bi + 1), :])
                    nc.vector.tensor_tensor(
                        A_sb[:, bi * N:(bi + 1) * N], art[:], ct[:], op=add)

            # consts
            I2_sb = cp.tile([128, 2048], dt)
            EI_sb = cp.tile([128, 2048], dt)
            ID_sb = cp.tile([128, 128], dt)
            B_sb = cp.tile([128, 4 * MD], dt)
            ones_col = cp.tile([128, 1], dt)
            ones_row = cp.tile([1, 128], dt)
            nc.sync.dma_start(I2_sb[:], I2_d[:])
            nc.sync.dma_start(EI_sb[:], EI_d[:])
            nc.sync.dma_start(ID_sb[:], ID_d[:])
            nc.sync.dma_start(B_sb[:], B_d[:])
            nc.vector.memset(ones_col[:], 1.0)
            nc.vector.memset(ones_row[:], 1.0)

            with tc.tile_pool(name="psumA", bufs=4, space="PSUM") as pp, \
                 tc.tile_pool(name="psumB", bufs=2, space="PSUM") as ppb, \
                 tc.tile_pool(name="symsc", bufs=1) as symp:


                def symmetrize(T_sb, scratch_tag):
                    """T <- (T + T^T)/2 for a block-layout (128,2048) tile."""
                    Th = symp.tile([128, 2048], dt, tag="Th")
                    for bj in range(NB):
                        ptr = ppb.tile([128, N], dt, tag="ptr")
                        for bi in range(NB):
                            nc.tensor.transpose(
                                ptr[:, bi * 128:(bi + 1) * 128],
                                T_sb[:, (bi * NB + bj) * 128:(bi * NB + bj + 1) * 128],
                                ID_sb[:])
                        nc.scalar.activation(
                            Th[:, bj * N:(bj + 1) * N], ptr[:],
                            mybir.ActivationFunctionType.Copy, scale=0.5)
                    nc.vector.scalar_tensor_tensor(
                        T_sb[:], T_sb[:], 0.5, Th[:], op0=mult, op1=add)

                # ------------ Phase N: c2 = 1/(MARGIN2 * smax_est^2) ------
                junk = work.tile([128, 2048], dt, tag="Xt")
                rowsq = sp.tile([128, 1], dt, tag="rowsq")
                nc.vector.scalar_tensor_tensor(
                    junk[:], A_sb[:], 1.0, A_sb[:], op0=mult, op1=mult,
                    accum_out=rowsq[:])
                pfro = ppb.tile([128, 512], dt, tag="ps_small")
                nc.tensor.matmul(pfro[0:1, 0:1], lhsT=ones_col[:], rhs=rowsq[:],
                                 start=True, stop=True)
                fro2 = sp.tile([1, 1], dt, tag="fro2")
                nc.scalar.activation(fro2[:], pfro[0:1, 0:1],
                                     mybir.ActivationFunctionType.Copy,
                                     scale=float(MARGIN2 * 4.0 / N))
                c2s = sp.tile([1, 1], dt, tag="c2s")
                nc.vector.reciprocal(c2s[:], fro2[:])
                pbc = ppb.tile([128, 512], dt, tag="ps_small")
                nc.tensor.matmul(pbc[:, 0:1], lhsT=ones_row[:], rhs=c2s[:],
                                 start=True, stop=True)
                c2_bc = sp.tile([128, 1], dt, tag="c2bc")
                nc.scalar.copy(c2_bc[:], pbc[:, 0:1])

                # ---------------- Phase T: At = A^T ----------------
                At_sb = mats.tile([128, 2048], dt, tag="At")
                for bj in range(NB):
                    ptr = ppb.tile([128, N], dt, tag="ptr")
                    for bi in range(NB):
                        nc.tensor.transpose(
                            ptr[:, bi * 128:(bi + 1) * 128],
                            A_sb[:, (bi * NB + bj) * 128:(bi * NB + bj + 1) * 128],
                            ID_sb[:])
                    nc.scalar.copy(At_sb[:, bj * N:(bj + 1) * N], ptr[:])

                # ---------------- Phase U0 / Xt0 ----------------
                Xt_sb = work.tile([128, 2048], dt, tag="Xt")
                nc.vector.tensor_scalar_mul(Xt_sb[:], A_sb[:], c2_bc[:])
                U_sb = work.tile([128, 2048], dt, tag="U")
                for m in range(NB):
                    pm = pp.tile([128, N], dt, tag="pmm")
                    for k in range(NB):
                        nc.tensor.matmul(
                            pm[:],
                            lhsT=At_sb[:, (k * NB + m) * 128:(k * NB + m + 1) * 128],
                            rhs=At_sb[:, k * N:(k + 1) * N],
                            start=(k == 0), stop=(k == NB - 1))
                    nc.vector.scalar_tensor_tensor(
                        U_sb[:, m * N:(m + 1) * N], pm[:], c2_bc[:],
                        EI_sb[:, m * N:(m + 1) * N], op0=mult, op1=add)

                # ---------------- Phase NS iterations ----------------
                for g in gammas:
                    gf = float(g)
                    P_sb = work.tile([128, 2048], dt, tag="P")
                    nc.vector.scalar_tensor_tensor(
                        P_sb[:], U_sb[:], -gf, I2_sb[:], op0=mult, op1=add)
                    Un = work.tile([128, 2048], dt, tag="U")
                    for m in range(NB):
                        pm = pp.tile([128, N], dt, tag="pmm")
                        for k in range(NB):
                            nc.tensor.matmul(
                                pm[:],
                                lhsT=U_sb[:, (k * NB + m) * 128:(k * NB + m + 1) * 128],
                                rhs=P_sb[:, k * N:(k + 1) * N],
                                start=(k == 0), stop=(k == NB - 1))
                        nc.vector.scalar_tensor_tensor(
                            Un[:, m * N:(m + 1) * N], pm[:], gf,
                            EI_sb[:, m * N:(m + 1) * N], op0=mult, op1=add)
                    Xn = work.tile([128, 2048], dt, tag="Xt")
                    for m in range(NB):
                        pm = pp.tile([128, N], dt, tag="pmm")
                        for k in range(NB):
                            nc.tensor.matmul(
                                pm[:],
                                lhsT=P_sb[:, (k * NB + m) * 128:(k * NB + m + 1) * 128],
                                rhs=Xt_sb[:, k * N:(k + 1) * N],
                                start=(k == 0), stop=(k == NB - 1))
                        nc.vector.tensor_scalar_mul(
                            Xn[:, m * N:(m + 1) * N], pm[:], gf)
                    symmetrize(Un, "Usym")
                    U_sb, Xt_sb = Un, Xn

                # ------------- Phase V: X = Xt^T; V = A@X -------------
                X_sb = mats.tile([128, 2048], dt, tag="A")
                for bj in range(NB):
                    ptr = ppb.tile([128, N], dt, tag="ptr")
                    for bi in range(NB):
                        nc.tensor.transpose(
                            ptr[:, bi * 128:(bi + 1) * 128],
                            Xt_sb[:, (bi * NB + bj) * 128:(bi * NB + bj + 1) * 128],
                            ID_sb[:])
                    nc.scalar.copy(X_sb[:, bj * N:(bj + 1) * N], ptr[:])
                V_sb = mats.tile([128, 2048], dt, tag="V")
                for m in range(NB):
                    pm = pp.tile([128, N], dt, tag="pmm")
                    for k in range(NB):
                        nc.tensor.matmul(
                            pm[:],
                            lhsT=At_sb[:, (k * NB + m) * 128:(k * NB + m + 1) * 128],
                            rhs=X_sb[:, k * N:(k + 1) * N],
                            start=(k == 0), stop=(k == NB - 1))
                    nc.scalar.copy(V_sb[:, m * N:(m + 1) * N], pm[:])
                Vt_sb = mats.tile([128, 2048], dt, tag="At")
                for bj in range(NB):
                    ptr = ppb.tile([128, N], dt, tag="ptr")
                    for bi in range(NB):
                        nc.tensor.transpose(
                            ptr[:, bi * 128:(bi + 1) * 128],
                            V_sb[:, (bi * NB + bj) * 128:(bi * NB + bj + 1) * 128],
                            ID_sb[:])
                    nc.scalar.copy(Vt_sb[:, bj * N:(bj + 1) * N], ptr[:])

                if dbg:
                    nc.sync.dma_start(dbg_d["dbg_c2"][:], c2_bc[:])
                    nc.sync.dma_start(dbg_d["dbg_U"][:], U_sb[:])
                    nc.sync.dma_start(dbg_d["dbg_Xt"][:], Xt_sb[:])
                    nc.sync.dma_start(dbg_d["dbg_V"][:], V_sb[:])
                    nc.sync.dma_start(dbg_d["dbg_At"][:], At_sb[:])

                # ---------------- Phase CG: solve V z = B ----------------
                W = 4 * MD  # 48

                def preduce(src48, dst12, tagbase):
                    pr = ppb.tile([128, 512], dt, tag="ps_small")
                    nc.tensor.matmul(pr[0:1, 0:W], lhsT=ones_col[:],
                                     rhs=src48[:], start=True, stop=True)
                    d48 = sp.tile([1, W], dt, tag="d48_" + tagbase)
                    nc.scalar.copy(d48[:], pr[0:1, 0:W])
                    nc.vector.tensor_tensor(d48[:, 0:MD], d48[:, 0:MD],
                                            d48[:, MD:2 * MD], op=add)
                    nc.vector.tensor_tensor(d48[:, 2 * MD:3 * MD],
                                            d48[:, 2 * MD:3 * MD],
                                            d48[:, 3 * MD:4 * MD], op=add)
                    nc.vector.tensor_tensor(dst12[:], d48[:, 0:MD],
                                            d48[:, 2 * MD:3 * MD], op=add)

                def bcast12(src12, dst48):
                    a48 = sp.tile([1, W], dt, tag="a48")
                    for kk in range(NB):
                        nc.vector.tensor_copy(a48[:, kk * MD:(kk + 1) * MD],
                                              src12[:])
                    pb2 = ppb.tile([128, 512], dt, tag="ps_small")
                    nc.tensor.matmul(pb2[:, 0:W], lhsT=ones_row[:], rhs=a48[:],
                                     start=True, stop=True)
                    nc.scalar.copy(dst48[:], pb2[:, 0:W])

                Z_t = sp.tile([128, W], dt, tag="Z")
                R_t = sp.tile([128, W], dt, tag="R")
                Pc_t = sp.tile([128, W], dt, tag="Pc")
                rs_t = sp.tile([1, MD], dt, tag="rs")
                nc.vector.memset(Z_t[:], 0.0)
                nc.vector.tensor_copy(R_t[:], B_sb[:])
                nc.vector.tensor_copy(Pc_t[:], B_sb[:])
                tt = sp.tile([128, W], dt, tag="tt")
                nc.vector.tensor_tensor(tt[:], R_t[:], R_t[:], op=mult)
                preduce(tt, rs_t, "rs")

                for it in range(CG_ITERS):
                    pv = ppb.tile([128, 512], dt, tag="ps_small")
                    for m in range(NB):
                        for k in range(NB):
                            nc.tensor.matmul(
                                pv[:, m * MD:(m + 1) * MD],
                                lhsT=Vt_sb[:, (k * NB + m) * 128:(k * NB + m + 1) * 128],
                                rhs=Pc_t[:, k * MD:(k + 1) * MD],
                                start=(k == 0), stop=(k == NB - 1))
                    Vp = sp.tile([128, W], dt, tag="Vp")
                    nc.scalar.copy(Vp[:], pv[:, 0:W])
                    nc.vector.tensor_tensor(tt[:], Pc_t[:], Vp[:], op=mult)
                    den = sp.tile([1, MD], dt, tag="den")
                    preduce(tt, den, "den")
                    rd = sp.tile([1, MD], dt, tag="rd")
                    nc.vector.reciprocal(rd[:], den[:])
                    alpha = sp.tile([1, MD], dt, tag="alpha")
                    nc.vector.tensor_tensor(alpha[:], rs_t[:], rd[:], op=mult)
                    ab = sp.tile([128, W], dt, tag="ab")
                    bcast12(alpha, ab)
                    nc.vector.tensor_tensor(tt[:], ab[:], Pc_t[:], op=mult)
                    nc.vector.tensor_tensor(Z_t[:], Z_t[:], tt[:], op=add)
                    nc.vector.tensor_tensor(tt[:], ab[:], Vp[:], op=mult)
                    nc.vector.tensor_tensor(R_t[:], R_t[:], tt[:], op=sub)
                    if it == CG_ITERS - 1:
                        break
                    nc.vector.tensor_tensor(tt[:], R_t[:], R_t[:], op=mult)
                    rsn = sp.tile([1, MD], dt, tag="rsn")
                    preduce(tt, rsn, "rsn")
                    rr = sp.tile([1, MD], dt, tag="rr")
                    nc.vector.reciprocal(rr[:], rs_t[:])
                    beta = sp.tile([1, MD], dt, tag="beta")
                    nc.vector.tensor_tensor(beta[:], rsn[:], rr[:], op=mult)
                    nc.vector.tensor_copy(rs_t[:], rsn[:])
                    bb = sp.tile([128, W], dt, tag="bb")
                    bcast12(beta, bb)
                    nc.vector.tensor_tensor(tt[:], bb[:], Pc_t[:], op=mult)
                    nc.vector.tensor_tensor(Pc_t[:], R_t[:], tt[:], op=add)

                if dbg:
                    nc.sync.dma_start(dbg_d["dbg_Z"][:], Z_t[:])
                    nc.sync.dma_start(dbg_d["dbg_R"][:], R_t[:])
                    nc.sync.dma_start(dbg_d["dbg_Pc"][:], Pc_t[:])

                # ---------------- Phase Y: Y = X @ z ----------------
                Y_sb = sp.tile([128, W], dt, tag="Y")
                pY = ppb.tile([128, 512], dt, tag="ps_small")
                for m in range(NB):
                    for k in range(NB):
                        nc.tensor.matmul(
                            pY[:, m * MD:(m + 1) * MD],
                            lhsT=Xt_sb[:, (k * NB + m) * 128:(k * NB + m + 1) * 128],
                            rhs=Z_t[:, k * MD:(k + 1) * MD],
                            start=(k == 0), stop=(k == NB - 1))
                nc.scalar.copy(Y_sb[:], pY[:, 0:W])

                # ---------------- Phase OUT: out = Y @ e ----------------
                Yt_sb = sp.tile([MD, N], dt, tag="Yt")
                pyt = ppb.tile([128, 512], dt, tag="ps_small")
                for m in range(NB):
                    nc.tensor.transpose(pyt[0:MD, m * 128:(m + 1) * 128],
                                        Y_sb[:, m * MD:(m + 1) * MD], ID_sb[:])
                nc.scalar.copy(Yt_sb[:], pyt[0:MD, :])
                e_sb = sp.tile([MD, JSH], dt, tag="e")
                nc.sync.dma_start(e_sb[:], e_d[:])
                for mi in range(NB):
                    po = pp.tile([128, N], dt, tag="pmm")
                    nc.tensor.matmul(po[:],
                                     lhsT=Yt_sb[:, mi * 128:(mi + 1) * 128],
                                     rhs=e_sb[:], start=True, stop=True)
                    ot = work.tile([128, JSH], dt, tag="ot")
                    nc.scalar.copy(ot[:], po[:])
                    nc.sync.dma_start(out_d[mi * 128:(mi + 1) * 128, :], ot[:])

    nc.compile()
    return nc


_NC_CACHE = None


def _get_nc():
    global _NC_CACHE
    if _NC_CACHE is None:
        _NC_CACHE = _build()
    return _NC_CACHE


def make_in_maps(M, r, constant_part, x):
    M = np.ascontiguousarray(M, dtype=_F)
    r = np.ascontiguousarray(r, dtype=_F)
    C = np.ascontiguousarray(constant_part, dtype=_F)
    e_full = np.ascontiguousarray(x, dtype=_F).reshape(MD, NPTS)
    I2 = _block_layout(2.0 * np.eye(N, dtype=_F))
    EI = _block_layout(np.float32(ETA) * np.eye(N, dtype=_F))
    ID = np.eye(128, dtype=_F)
    B = _vec_layout(np.eye(N, dtype=_F)[:, N - MD:])
    in_maps = []
    for c in range(NCORES):
        in_maps.append({
            "M": np.ascontiguousarray(M[c * ESH:(c + 1) * ESH]),
            "r_rep": np.ascontiguousarray(
                np.repeat(r[c * ESH:(c + 1) * ESH, None], 32, axis=1)),
            "C": C,
            "e": np.ascontiguousarray(e_full[:, c * JSH:(c + 1) * JSH]),
            "I2": I2, "EI": EI, "ID": ID, "B": B,
        })
    return in_maps


def kernel(M, r, constant_part, x):
    from concourse.bass_utils import run_bass_kernel_spmd
    nc = _get_nc()
    in_maps = make_in_maps(M, r, constant_part, x)
    res = run_bass_kernel_spmd(nc, in_maps, core_ids=list(range(NCORES)))
    out = np.concatenate([res.results[c]["out"] for c in range(NCORES)], axis=1)
    return np.ascontiguousarray(out, dtype=_F)
